# revision 13
# baseline (speedup 1.0000x reference)
"""Trainium2 Bass kernel: CombinedModel = DNN branch (Linear+BatchNorm+ReLU)
+ GCN branch (2x GCNConv -> mean pool) + linear head, on 8 NeuronCores.

Strategy (all FLOPs on float inputs run on-device):
- GCN layer 1: edges (incl. self-loops, sorted by destination) sharded by
  destination range across cores; per 128-edge tile, gather source x2 rows
  with dma_gather, build one-hot (dest) selection matrices on DVE, and
  scatter-accumulate on the TensorEngine into per-128-dest-block PSUM.
  aggX @ W1 (+b1, ReLU) per block.
- GCN layer 2 + mean pool collapse algebraically: only mean(out2) is needed,
  so sum_c out2[c] = (sum_n wslf[n]*g1[n]) @ W2 + N*b2, where wslf depends on
  the graph indices only (host-computed). Each core reduces its own dest
  shard; a single tiny AllReduce combines [bn_sum | bn_sumsq | p] stats.
- DNN branch: batch-sharded, computed transposed (hT = dnn_W^T @ x1^T) so
  BatchNorm scale/shift are per-partition; dnn_b cancels exactly in BN.
- Head: no nonlinearity between fc1/fc2 -> fold into out = dnn_emb @ v + s0
  with v = fc1_W[:64] @ fc2_W and s0 a scalar from the pooled GNN embedding.
"""

import math
import os
import sys

for _p in ("/opt/trn_rl_repo", "/root/.axon_site/_ro/trn_rl_repo"):
    if os.path.isdir(_p) and _p not in sys.path:
        sys.path.append(_p)

import numpy as np

import concourse.bacc as bacc
import concourse.bass as bass
import concourse.mybir as mybir
import concourse.tile as tile
from concourse import bass_utils
from concourse.masks import make_identity

DT = mybir.dt
ALU = mybir.AluOpType
ACTF = mybir.ActivationFunctionType

N_NODES = 50000
N_EDGES = 800000
BATCH = 16384
DNN_IN = 256
F = 64                       # feature width everywhere in the GNN
CORES = 8
NSH = N_NODES // CORES       # 6250 dest nodes per core
BSH = BATCH // CORES         # 2048 batch rows per core
NBLK = (NSH + 127) // 128    # 49 dest blocks per core
SPLIT = 32768                # int16 gather index limit
BN_EPS = 1e-5
CHUNK_BLK = 7                # dest blocks per gather chunk
GMAX_T = 8                   # max 128-edge tiles per dma_gather call (ucode cap)

# debug switches: selectively disable program phases when bisecting
_PHASES = dict(dnn=True, gcn=True, head=True, gather=True, consts=True)


def _cdiv(a, b):
    return (a + b - 1) // b


# --------------------------------------------------------------------------
# Host-side preprocessing: graph indices -> per-core packed gather/one-hot
# metadata with a core-uniform tile structure (SPMD requires one program).
# --------------------------------------------------------------------------

def _prep(inputs):
    x1 = np.asarray(inputs["x1"], np.float32)
    x2 = np.ascontiguousarray(np.asarray(inputs["x2"], np.float32))
    ei = np.asarray(inputs["edge_index"])
    row = ei[0].astype(np.int64)
    col = ei[1].astype(np.int64)

    deg = (np.bincount(col, minlength=N_NODES) + 1.0).astype(np.float32)
    dis = (1.0 / np.sqrt(deg)).astype(np.float32)
    norm = dis[row] * dis[col]

    # layer-2 collapse weights: sum_c out2[c] = sum_n wslf[n] * h2[n] + N*b2
    w_r = np.bincount(row, weights=dis[col].astype(np.float64), minlength=N_NODES)
    wslf = (dis * w_r.astype(np.float32) + dis * dis).astype(np.float32)

    # self-loops as ordinary edges with norm = dis^2
    ar = np.arange(N_NODES, dtype=np.int64)
    row2 = np.concatenate([row, ar])
    col2 = np.concatenate([col, ar])
    nrm2 = np.concatenate([norm, dis * dis]).astype(np.float32)

    order = np.argsort(col2, kind="stable")
    srow = row2[order]
    scol = col2[order]
    snrm = nrm2[order]

    # per (core, block) segments, split lo/hi by source < SPLIT
    segs = [[None] * NBLK for _ in range(CORES)]
    n_lo = np.zeros((CORES, NBLK), np.int64)
    n_hi = np.zeros((CORES, NBLK), np.int64)
    for k in range(CORES):
        base = k * NSH
        s0 = np.searchsorted(scol, base)
        s1 = np.searchsorted(scol, base + NSH)
        krow = srow[s0:s1]
        knrm = snrm[s0:s1]
        rel = scol[s0:s1] - base
        bst = np.searchsorted(rel, np.arange(NBLK) * 128)
        ben = np.append(bst[1:], rel.size)
        for b in range(NBLK):
            sl = slice(bst[b], ben[b])
            r = krow[sl]
            n = knrm[sl]
            c = (rel[sl] - b * 128).astype(np.float32)
            lo = r < SPLIT
            hi = ~lo
            segs[k][b] = (r[lo], n[lo], c[lo], r[hi] - SPLIT, n[hi], c[hi])
            n_lo[k, b] = int(lo.sum())
            n_hi[k, b] = int(hi.sum())

    T_LO = [int(_cdiv(int(n_lo[:, b].max()), 128)) for b in range(NBLK)]
    T_HI = [int(_cdiv(int(n_hi[:, b].max()), 128)) for b in range(NBLK)]
    for b in range(NBLK):
        assert T_LO[b] + T_HI[b] >= 1
    TLOS = sum(T_LO)
    THIS = sum(T_HI)

    def pack_stream(k, which):
        # concatenated per-block edge data padded to T[b]*128 entries
        T = T_LO if which == 0 else T_HI
        tot = sum(T) * 128
        idx = np.zeros(tot, np.int16)
        nrm = np.zeros(tot, np.float32)
        crl = np.zeros(tot, np.float32)
        off = 0
        for b in range(NBLK):
            s = segs[k][b]
            r, n, c = (s[0], s[1], s[2]) if which == 0 else (s[3], s[4], s[5])
            m = r.size
            idx[off:off + m] = r.astype(np.int16)
            nrm[off:off + m] = n
            crl[off:off + m] = c
            off += T[b] * 128
        ntile = tot // 128
        # idx layout for dma_gather: [128, ntile*8] int16, idx i at
        # [i % 16 + 16*rep, i // 16], replicated across the 8 Q7 cores
        idx16 = np.ascontiguousarray(np.tile(idx.reshape(-1, 16).T, (8, 1)))
        nrm_t = np.ascontiguousarray(nrm.reshape(ntile, 128).T)
        crl_t = np.ascontiguousarray(crl.reshape(ntile, 128).T)
        return idx16, nrm_t, crl_t

    per_core = []
    x1t_full = np.ascontiguousarray(x1.T)
    for k in range(CORES):
        ilo, nlo, clo = pack_stream(k, 0)
        ihi, nhi, chi = pack_stream(k, 1)
        wk = np.zeros(NBLK * 128, np.float32)
        wk[:NSH] = wslf[k * NSH:(k + 1) * NSH]
        per_core.append(dict(
            idxlo=ilo, nrmlo=nlo, crllo=clo,
            idxhi=ihi, nrmhi=nhi, crlhi=chi,
            wslf=np.ascontiguousarray(wk.reshape(NBLK, 128).T),
            x1t=np.ascontiguousarray(x1t_full[:, k * BSH:(k + 1) * BSH]),
        ))

    # host-folded head weights (no nonlinearity between fc1 and fc2)
    fc1 = np.asarray(inputs["fc1_W"], np.float32)
    fc2 = np.asarray(inputs["fc2_W"], np.float32)
    u = fc1[F:, :] @ fc2                                    # [64, 1]
    v = np.ascontiguousarray(fc1[:F, :] @ fc2)              # [64, 1]
    z = np.ascontiguousarray(np.asarray(inputs["gcn2_W"], np.float32) @ u)
    c1 = float(np.asarray(inputs["fc1_b"], np.float32) @ fc2[:, 0]
               + np.asarray(inputs["fc2_b"], np.float32)[0]
               + np.asarray(inputs["gcn2_b"], np.float32) @ u[:, 0])

    shared = dict(
        x2=x2,
        w1=np.ascontiguousarray(np.asarray(inputs["gcn1_W"], np.float32)),
        b1b=np.ascontiguousarray(
            np.tile(np.asarray(inputs["gcn1_b"], np.float32), (128, 1))),
        dnnw=np.ascontiguousarray(np.asarray(inputs["dnn_W"], np.float32)),
        gma=np.ascontiguousarray(
            np.asarray(inputs["bn_gamma"], np.float32).reshape(F, 1)),
        bta=np.ascontiguousarray(
            np.asarray(inputs["bn_beta"], np.float32).reshape(F, 1)),
        vc=v, zc=z,
        iota=np.ascontiguousarray(
            np.broadcast_to(np.arange(128, dtype=np.float32), (128, 128))),
    )
    return dict(T_LO=T_LO, T_HI=T_HI, TLOS=TLOS, THIS=THIS, c1=c1,
                per_core=per_core, shared=shared)


# --------------------------------------------------------------------------
# Device program
# --------------------------------------------------------------------------

def _build_program(T_LO, T_HI, c1, reps=1):
    TLOS, THIS = sum(T_LO), sum(T_HI)
    nc = bacc.Bacc("TRN2", target_bir_lowering=False, debug=False,
                   enable_asserts=False, num_devices=CORES)
    ap = {}

    def inp(name, shape, dt=DT.float32):
        ap[name] = nc.dram_tensor(name, list(shape), dt,
                                  kind="ExternalInput").ap()

    inp("x2", (N_NODES, F))
    inp("x1t", (DNN_IN, BSH))
    inp("idxlo", (128, max(TLOS, 1) * 8), DT.int16)
    inp("idxhi", (128, max(THIS, 1) * 8), DT.int16)
    inp("nrmlo", (128, max(TLOS, 1)))
    inp("crllo", (128, max(TLOS, 1)))
    inp("nrmhi", (128, max(THIS, 1)))
    inp("crlhi", (128, max(THIS, 1)))
    inp("wslf", (128, NBLK))
    inp("w1", (F, F))
    inp("b1b", (128, F))
    inp("dnnw", (DNN_IN, F))
    inp("gma", (F, 1))
    inp("bta", (F, 1))
    inp("vc", (F, 1))
    inp("zc", (F, 1))
    inp("iota", (128, 128))
    out_ap = nc.dram_tensor("out", [1, BSH], DT.float32,
                            kind="ExternalOutput").ap()

    # chunk layout: groups of CHUNK_BLK dest blocks share one gather pair
    lo_off = np.concatenate([[0], np.cumsum(T_LO)]).astype(int)
    hi_off = np.concatenate([[0], np.cumsum(T_HI)]).astype(int)
    chunks = []
    for c0 in range(0, NBLK, CHUNK_BLK):
        c1b = min(c0 + CHUNK_BLK, NBLK)
        chunks.append(dict(
            blocks=list(range(c0, c1b)),
            lo_t0=int(lo_off[c0]), lo_t1=int(lo_off[c1b]),
            hi_t0=int(hi_off[c0]), hi_t1=int(hi_off[c1b]),
        ))
    max_lo = max(ch["lo_t1"] - ch["lo_t0"] for ch in chunks)
    max_hi = max(ch["hi_t1"] - ch["hi_t0"] for ch in chunks)

    with tile.TileContext(nc) as tc:
        with tc.tile_pool(name="const", bufs=1) as cp, \
             tc.tile_pool(name="dram", bufs=1, space="DRAM") as dp:
            # ---- load constants / per-core packed inputs once ----
            def load(name, shape, dt=DT.float32, src=None):
                t = cp.tile(list(shape), dt, tag=name)
                nc.sync.dma_start(out=t[:], in_=src if src is not None
                                  else ap[name][:])
                return t

            w1_sb = load("w1", (F, F))
            b1b_sb = load("b1b", (128, F))
            gma_sb = load("gma", (F, 1))
            bta_sb = load("bta", (F, 1))
            vc_sb = load("vc", (F, 1))
            zc_sb = load("zc", (F, 1))
            iota_sb = load("iota", (128, 128))
            wslf_sb = load("wslf", (128, NBLK))
            dnnw0 = load("dnnw0", (128, F), src=ap["dnnw"][0:128, :])
            dnnw1 = load("dnnw1", (128, F), src=ap["dnnw"][128:256, :])
            x1a = load("x1a", (128, BSH), src=ap["x1t"][0:128, :])
            x1b = load("x1b", (128, BSH), src=ap["x1t"][128:256, :])
            idxlo_sb = load("idxlo", (128, max(TLOS, 1) * 8), DT.int16)
            idxhi_sb = load("idxhi", (128, max(THIS, 1) * 8), DT.int16)
            nrmlo_sb = load("nrmlo", (128, max(TLOS, 1)))
            crllo_sb = load("crllo", (128, max(TLOS, 1)))
            nrmhi_sb = load("nrmhi", (128, max(THIS, 1)))
            crlhi_sb = load("crlhi", (128, max(THIS, 1)))

            ident = cp.tile([128, 128], DT.float32, tag="ident")
            make_identity(nc, ident[:])

            # persistent accumulators / phase outputs
            hT = cp.tile([F, BSH], DT.float32, tag="hT")
            sqtmp = cp.tile([F, BSH], DT.float32, tag="sqtmp")
            bn_sum = cp.tile([F, 1], DT.float32, tag="bn_sum")
            bn_sq = cp.tile([F, 1], DT.float32, tag="bn_sq")
            p_acc = cp.tile([F, 1], DT.float32, tag="p_acc")

            def phase_ab():
                if _PHASES["dnn"]:
                    _dnn_phase()
                else:
                    nc.vector.memset(hT[:], 0.0)
                    nc.vector.memset(bn_sum[:], 0.0)
                    nc.vector.memset(bn_sq[:], 1.0)
                    nc.vector.memset(sqtmp[:], 0.0)
                if _PHASES["gcn"]:
                    _gcn_phase()
                else:
                    nc.vector.memset(p_acc[:], 0.0)

            def _dnn_phase():
                with tc.tile_pool(name="pd", bufs=1, space="PSUM") as pd:
                    for c in range(BSH // 512):
                        ps = pd.tile([F, 512], DT.float32)
                        cs = slice(c * 512, (c + 1) * 512)
                        nc.tensor.matmul(out=ps[:], lhsT=dnnw0[:],
                                         rhs=x1a[:, cs], start=True, stop=False)
                        nc.tensor.matmul(out=ps[:], lhsT=dnnw1[:],
                                         rhs=x1b[:, cs], start=False, stop=True)
                        nc.vector.tensor_copy(out=hT[:, cs], in_=ps[:])
                nc.vector.reduce_sum(out=bn_sum[:], in_=hT[:],
                                     axis=mybir.AxisListType.X)
                nc.scalar.activation(out=sqtmp[:], in_=hT[:], func=ACTF.Square,
                                     accum_out=bn_sq[:])

            def _gcn_phase():
                # ---------------- GCN layer 1 ----------------
                nc.vector.memset(p_acc[:], 0.0)
                with tc.tile_pool(name="gp", bufs=2) as gp, \
                     tc.tile_pool(name="sp", bufs=4) as sp, \
                     tc.tile_pool(name="wp", bufs=3) as wp, \
                     tc.tile_pool(name="pa", bufs=2, space="PSUM") as pa, \
                     tc.tile_pool(name="pt", bufs=1, space="PSUM") as pt, \
                     tc.tile_pool(name="po", bufs=2, space="PSUM") as po, \
                     tc.tile_pool(name="pb", bufs=2, space="PSUM") as pb:
                    for ch in chunks:
                        nlo = ch["lo_t1"] - ch["lo_t0"]
                        nhi = ch["hi_t1"] - ch["hi_t0"]
                        glo = ghi = None
                        if not _PHASES["gather"]:
                            glo = gp.tile([128, max_lo * F], DT.float32,
                                          tag="glo")
                            ghi = gp.tile([128, max_hi * F], DT.float32,
                                          tag="ghi")
                            nc.vector.memset(glo[:], 0.0)
                            nc.vector.memset(ghi[:], 0.0)
                        elif nlo:
                            # gather in <=GMAX_T-tile slices: the dma_gather
                            # ucode faults above ~1024 indices per call
                            glo = gp.tile([128, max_lo * F], DT.float32,
                                          tag="glo")
                            for s0 in range(0, nlo, GMAX_T):
                                nt = min(GMAX_T, nlo - s0)
                                nc.gpsimd.dma_gather(
                                    out_ap=glo[:, s0 * F:(s0 + nt) * F]
                                    .rearrange("p (t e) -> p t e", e=F),
                                    in_ap=ap["x2"][0:SPLIT, :],
                                    idxs_ap=idxlo_sb[
                                        :, (ch["lo_t0"] + s0) * 8:
                                        (ch["lo_t0"] + s0 + nt) * 8],
                                    num_idxs=nt * 128,
                                    num_idxs_reg=nt * 128,
                                    elem_size=F,
                                )
                        if nhi and _PHASES["gather"]:
                            ghi = gp.tile([128, max_hi * F], DT.float32,
                                          tag="ghi")
                            for s0 in range(0, nhi, GMAX_T):
                                nt = min(GMAX_T, nhi - s0)
                                nc.gpsimd.dma_gather(
                                    out_ap=ghi[:, s0 * F:(s0 + nt) * F]
                                    .rearrange("p (t e) -> p t e", e=F),
                                    in_ap=ap["x2"][SPLIT:N_NODES, :],
                                    idxs_ap=idxhi_sb[
                                        :, (ch["hi_t0"] + s0) * 8:
                                        (ch["hi_t0"] + s0 + nt) * 8],
                                    num_idxs=nt * 128,
                                    num_idxs_reg=nt * 128,
                                    elem_size=F,
                                )
                        for b in ch["blocks"]:
                            tl = []
                            for t in range(T_LO[b]):
                                tl.append(("lo", lo_off[b] - ch["lo_t0"] + t,
                                           lo_off[b] + t))
                            for t in range(T_HI[b]):
                                tl.append(("hi", hi_off[b] - ch["hi_t0"] + t,
                                           hi_off[b] + t))
                            agg = pa.tile([128, F], DT.float32)
                            for ti, (src, lt, gt) in enumerate(tl):
                                g = glo if src == "lo" else ghi
                                nrm = nrmlo_sb if src == "lo" else nrmhi_sb
                                crl = crllo_sb if src == "lo" else crlhi_sb
                                stile = sp.tile([128, 128], DT.float32,
                                                tag="stile")
                                nc.vector.tensor_scalar(
                                    out=stile[:], in0=iota_sb[:],
                                    scalar1=crl[:, gt:gt + 1],
                                    scalar2=nrm[:, gt:gt + 1],
                                    op0=ALU.is_equal, op1=ALU.mult)
                                nc.tensor.matmul(
                                    out=agg[:], lhsT=stile[:],
                                    rhs=g[:, lt * F:(lt + 1) * F],
                                    start=(ti == 0), stop=(ti == len(tl) - 1))
                            aggsb = wp.tile([128, F], DT.float32, tag="aggsb")
                            nc.vector.tensor_copy(out=aggsb[:], in_=agg[:])
                            pst = pt.tile([F, 128], DT.float32)
                            nc.tensor.transpose(out=pst[:], in_=aggsb[:],
                                                identity=ident[:])
                            aggT = wp.tile([F, 128], DT.float32, tag="aggT")
                            nc.vector.tensor_copy(out=aggT[:], in_=pst[:])
                            o1 = po.tile([128, F], DT.float32)
                            nc.tensor.matmul(out=o1[:], lhsT=aggT[:],
                                             rhs=w1_sb[:], start=True,
                                             stop=True)
                            g1 = wp.tile([128, F], DT.float32, tag="g1")
                            nc.vector.tensor_tensor(out=g1[:], in0=o1[:],
                                                    in1=b1b_sb[:], op=ALU.add)
                            nc.scalar.activation(out=g1[:], in_=g1[:],
                                                 func=ACTF.Relu)
                            pbt = pb.tile([F, 1], DT.float32)
                            nc.tensor.matmul(out=pbt[:], lhsT=g1[:],
                                             rhs=wslf_sb[:, b:b + 1],
                                             start=True, stop=True)
                            nc.vector.tensor_tensor(out=p_acc[:], in0=p_acc[:],
                                                    in1=pbt[:], op=ALU.add)

            if reps == 1:
                phase_ab()
            else:
                with tc.For_i(0, reps, 1):
                    phase_ab()

            # ---------------- cross-core stats + head ----------------
            stats = cp.tile([F, 4], DT.float32, tag="stats")
            nc.vector.tensor_copy(out=stats[:, 0:1], in_=bn_sum[:])
            nc.vector.tensor_copy(out=stats[:, 1:2], in_=bn_sq[:])
            nc.vector.tensor_copy(out=stats[:, 2:3], in_=p_acc[:])
            cc_in = dp.tile([F, 3], DT.float32)
            cc_out = dp.tile([F, 3], DT.float32)
            nc.gpsimd.dma_start(out=cc_in[:], in_=stats[:, 0:3])
            nc.gpsimd.collective_compute(
                "AllReduce", ALU.add,
                replica_groups=[list(range(CORES))],
                ins=[cc_in.opt()], outs=[cc_out.opt()],
            )
            tot = cp.tile([F, 3], DT.float32, tag="tot")
            nc.sync.dma_start(out=tot[:], in_=cc_out[:])

            if not _PHASES["head"]:
                outsb0 = cp.tile([1, BSH], DT.float32, tag="outsb")
                nc.vector.memset(outsb0[:], 0.0)
                nc.sync.dma_start(out=out_ap[:], in_=outsb0[:])
            if _PHASES["head"]:
              with tc.tile_pool(name="pc", bufs=2, space="PSUM") as pc:
                sm = cp  # small [64,1] scratch tiles live in the const pool
                mu = sm.tile([F, 1], DT.float32, tag="mu")
                nc.vector.tensor_scalar(out=mu[:], in0=tot[:, 0:1],
                                        scalar1=1.0 / BATCH, scalar2=None,
                                        op0=ALU.mult)
                ex2 = sm.tile([F, 1], DT.float32, tag="ex2")
                nc.vector.tensor_scalar(out=ex2[:], in0=tot[:, 1:2],
                                        scalar1=1.0 / BATCH, scalar2=None,
                                        op0=ALU.mult)
                m2 = sm.tile([F, 1], DT.float32, tag="m2")
                nc.vector.tensor_tensor(out=m2[:], in0=mu[:], in1=mu[:],
                                        op=ALU.mult)
                var = sm.tile([F, 1], DT.float32, tag="var")
                nc.vector.tensor_tensor(out=var[:], in0=ex2[:], in1=m2[:],
                                        op=ALU.subtract)
                vp = sm.tile([F, 1], DT.float32, tag="vp")
                nc.vector.tensor_scalar(out=vp[:], in0=var[:],
                                        scalar1=BN_EPS, scalar2=None,
                                        op0=ALU.add)
                sd = sm.tile([F, 1], DT.float32, tag="sd")
                nc.scalar.activation(out=sd[:], in_=vp[:], func=ACTF.Sqrt)
                istd = sm.tile([F, 1], DT.float32, tag="istd")
                nc.vector.reciprocal(out=istd[:], in_=sd[:])
                scl = sm.tile([F, 1], DT.float32, tag="scl")
                nc.vector.tensor_tensor(out=scl[:], in0=istd[:], in1=gma_sb[:],
                                        op=ALU.mult)
                msc = sm.tile([F, 1], DT.float32, tag="msc")
                nc.vector.tensor_tensor(out=msc[:], in0=mu[:], in1=scl[:],
                                        op=ALU.mult)
                shf = sm.tile([F, 1], DT.float32, tag="shf")
                nc.vector.tensor_tensor(out=shf[:], in0=bta_sb[:], in1=msc[:],
                                        op=ALU.subtract)
                # dnn_embT = relu(hT * scale + shift), in place
                nc.scalar.activation(out=hT[:], in_=hT[:], func=ACTF.Relu,
                                     scale=scl[:, :], bias=shf[:, :])
                # s0 = (p . z)/N + c1
                s0p = pc.tile([1, 1], DT.float32, tag="s0p")
                nc.tensor.matmul(out=s0p[:], lhsT=zc_sb[:], rhs=tot[:, 2:3],
                                 start=True, stop=True)
                s0 = sm.tile([1, 1], DT.float32, tag="s0")
                nc.vector.tensor_scalar(out=s0[:], in0=s0p[:],
                                        scalar1=1.0 / N_NODES, scalar2=c1,
                                        op0=ALU.mult, op1=ALU.add)
                outsb = cp.tile([1, BSH], DT.float32, tag="outsb")
                for c in range(BSH // 512):
                    cs = slice(c * 512, (c + 1) * 512)
                    pov = pc.tile([1, 512], DT.float32, tag="pov")
                    nc.tensor.matmul(out=pov[:], lhsT=vc_sb[:], rhs=hT[:, cs],
                                     start=True, stop=True)
                    nc.vector.tensor_scalar(out=outsb[:, cs], in0=pov[:],
                                            scalar1=s0[:, :], scalar2=None,
                                            op0=ALU.add)
                nc.sync.dma_start(out=out_ap[:], in_=outsb[:])

    nc.compile()
    return nc


_CACHE = {}


def _get_program(T_LO, T_HI, c1, reps=1):
    key = (tuple(T_LO), tuple(T_HI), float(c1), reps)
    if key not in _CACHE:
        _CACHE[key] = _build_program(T_LO, T_HI, c1, reps)
    return _CACHE[key]


def _in_maps(st):
    maps = []
    for k in range(CORES):
        m = dict(st["shared"])
        m.update(st["per_core"][k])
        maps.append(m)
    return maps


def kernel(**inputs):
    st = _prep(inputs)
    nc = _get_program(st["T_LO"], st["T_HI"], st["c1"], reps=1)
    res = bass_utils.run_bass_kernel_spmd(
        nc, _in_maps(st), core_ids=list(range(CORES)))
    out = np.concatenate(
        [res.results[k]["out"].reshape(BSH, 1) for k in range(CORES)], axis=0)
    return out.astype(np.float32)


# revision 21
# speedup vs baseline: 11.4797x; 11.4797x over previous
"""Trainium2 Bass kernel: CombinedModel = DNN branch (Linear+BatchNorm+ReLU)
+ GCN branch (2x GCNConv -> mean pool) + linear head, on 8 NeuronCores.

Strategy (all FLOPs on float inputs run on-device):
- GCN layer 1: edges (incl. self-loops, sorted by destination) sharded by
  destination range across cores; per 128-edge tile, gather source x2 rows
  with dma_gather, build one-hot (dest) selection matrices on DVE, and
  scatter-accumulate on the TensorEngine into per-128-dest-block PSUM.
  aggX @ W1 (+b1, ReLU) per block.
- GCN layer 2 + mean pool collapse algebraically: only mean(out2) is needed,
  so sum_c out2[c] = (sum_n wslf[n]*g1[n]) @ W2 + N*b2, where wslf depends on
  the graph indices only (host-computed). Each core reduces its own dest
  shard; a single tiny AllReduce combines [bn_sum | bn_sumsq | p] stats.
- DNN branch: batch-sharded, computed transposed (hT = dnn_W^T @ x1^T) so
  BatchNorm scale/shift are per-partition; dnn_b cancels exactly in BN.
- Head: no nonlinearity between fc1/fc2 -> fold into out = dnn_emb @ v + s0
  with v = fc1_W[:64] @ fc2_W and s0 a scalar from the pooled GNN embedding.
"""

import math
import os
import sys

for _p in ("/opt/trn_rl_repo", "/root/.axon_site/_ro/trn_rl_repo"):
    if os.path.isdir(_p) and _p not in sys.path:
        sys.path.append(_p)

import numpy as np

import concourse.bacc as bacc
import concourse.bass as bass
import concourse.mybir as mybir
import concourse.tile as tile
from concourse import bass_utils
from concourse.masks import make_identity

DT = mybir.dt
ALU = mybir.AluOpType
ACTF = mybir.ActivationFunctionType

N_NODES = 50000
N_EDGES = 800000
BATCH = 16384
DNN_IN = 256
F = 64                       # feature width everywhere in the GNN
CORES = 8
NSH = N_NODES // CORES       # 6250 dest nodes per core
BSH = BATCH // CORES         # 2048 batch rows per core
NBLK = (NSH + 127) // 128    # 49 dest blocks per core
SPLIT = 32768                # int16 gather index limit
BN_EPS = 1e-5
CHUNK_BLK = 7                # dest blocks per gather chunk
GMAX_T = 1                   # tiles per indirect gather call: HW honors only
                             # one row-offset per partition per call

# debug switches: selectively disable program phases when bisecting
_PHASES = dict(dnn=True, gcn=True, head=True, gather=True, consts=True)


def _cdiv(a, b):
    return (a + b - 1) // b


# --------------------------------------------------------------------------
# Host-side preprocessing: graph indices -> per-core packed gather/one-hot
# metadata with a core-uniform tile structure (SPMD requires one program).
# --------------------------------------------------------------------------

def _prep(inputs):
    x1 = np.asarray(inputs["x1"], np.float32)
    x2 = np.ascontiguousarray(np.asarray(inputs["x2"], np.float32))
    ei = np.asarray(inputs["edge_index"])
    row = ei[0].astype(np.int64)
    col = ei[1].astype(np.int64)

    deg = (np.bincount(col, minlength=N_NODES) + 1.0).astype(np.float32)
    dis = (1.0 / np.sqrt(deg)).astype(np.float32)
    norm = dis[row] * dis[col]

    # layer-2 collapse weights: sum_c out2[c] = sum_n wslf[n] * h2[n] + N*b2
    w_r = np.bincount(row, weights=dis[col].astype(np.float64), minlength=N_NODES)
    wslf = (dis * w_r.astype(np.float32) + dis * dis).astype(np.float32)

    # self-loops as ordinary edges with norm = dis^2
    ar = np.arange(N_NODES, dtype=np.int64)
    row2 = np.concatenate([row, ar])
    col2 = np.concatenate([col, ar])
    nrm2 = np.concatenate([norm, dis * dis]).astype(np.float32)

    order = np.argsort(col2, kind="stable")
    srow = row2[order]
    scol = col2[order]
    snrm = nrm2[order]

    # per (core, block) segments; indirect DMA takes int32 row offsets, so
    # no index-range splitting is needed
    segs = [[None] * NBLK for _ in range(CORES)]
    n_e = np.zeros((CORES, NBLK), np.int64)
    for k in range(CORES):
        base = k * NSH
        s0 = np.searchsorted(scol, base)
        s1 = np.searchsorted(scol, base + NSH)
        krow = srow[s0:s1]
        knrm = snrm[s0:s1]
        rel = scol[s0:s1] - base
        bst = np.searchsorted(rel, np.arange(NBLK) * 128)
        ben = np.append(bst[1:], rel.size)
        for b in range(NBLK):
            sl = slice(bst[b], ben[b])
            r = krow[sl]
            n = knrm[sl]
            c = (rel[sl] - b * 128).astype(np.float32)
            # within a block the edge order is irrelevant to the sum; sort
            # by source row so the gather's HBM accesses are ascending
            o = np.argsort(r, kind="stable")
            segs[k][b] = (r[o], n[o], c[o])
            n_e[k, b] = ben[b] - bst[b]

    T_LO = [int(_cdiv(int(n_e[:, b].max()), 128)) for b in range(NBLK)]
    T_HI = [0] * NBLK
    for b in range(NBLK):
        assert T_LO[b] >= 1
    TLOS = sum(T_LO)
    THIS = 0

    def pack_stream(k):
        # concatenated per-block edge data padded to T[b]*128 entries
        tot = TLOS * 128
        idx = np.zeros(tot, np.int32)
        nrm = np.zeros(tot, np.float32)
        crl = np.zeros(tot, np.float32)
        off = 0
        for b in range(NBLK):
            r, n, c = segs[k][b]
            m = r.size
            idx[off:off + m] = r.astype(np.int32)
            nrm[off:off + m] = n
            crl[off:off + m] = c
            off += T_LO[b] * 128
        ntile = tot // 128
        # [128, T] layout: column t, partition p  <->  edge t*128 + p
        idx_t = np.ascontiguousarray(idx.reshape(ntile, 128).T)
        nrm_t = np.ascontiguousarray(nrm.reshape(ntile, 128).T)
        crl_t = np.ascontiguousarray(crl.reshape(ntile, 128).T)
        return idx_t, nrm_t, crl_t

    per_core = []
    x1t_full = np.ascontiguousarray(x1.T)
    for k in range(CORES):
        ilo, nlo, clo = pack_stream(k)
        wk = np.zeros(NBLK * 128, np.float32)
        wk[:NSH] = wslf[k * NSH:(k + 1) * NSH]
        per_core.append(dict(
            idx=ilo, nrm=nlo, crl=clo,
            wslf=np.ascontiguousarray(wk.reshape(NBLK, 128).T),
            x1t=np.ascontiguousarray(x1t_full[:, k * BSH:(k + 1) * BSH]),
        ))

    # host-folded head weights (no nonlinearity between fc1 and fc2)
    fc1 = np.asarray(inputs["fc1_W"], np.float32)
    fc2 = np.asarray(inputs["fc2_W"], np.float32)
    u = fc1[F:, :] @ fc2                                    # [64, 1]
    v = np.ascontiguousarray(fc1[:F, :] @ fc2)              # [64, 1]
    z = np.ascontiguousarray(np.asarray(inputs["gcn2_W"], np.float32) @ u)
    c1 = float(np.asarray(inputs["fc1_b"], np.float32) @ fc2[:, 0]
               + np.asarray(inputs["fc2_b"], np.float32)[0]
               + np.asarray(inputs["gcn2_b"], np.float32) @ u[:, 0])

    shared = dict(
        x2=x2,
        w1=np.ascontiguousarray(np.asarray(inputs["gcn1_W"], np.float32)),
        b1b=np.ascontiguousarray(
            np.tile(np.asarray(inputs["gcn1_b"], np.float32), (128, 1))),
        dnnw=np.ascontiguousarray(np.asarray(inputs["dnn_W"], np.float32)),
        gma=np.ascontiguousarray(
            np.asarray(inputs["bn_gamma"], np.float32).reshape(F, 1)),
        bta=np.ascontiguousarray(
            np.asarray(inputs["bn_beta"], np.float32).reshape(F, 1)),
        vc=v, zc=z,
        iota=np.ascontiguousarray(
            np.broadcast_to(np.arange(128, dtype=np.float32), (128, 128))),
    )
    return dict(T_LO=T_LO, T_HI=T_HI, TLOS=TLOS, THIS=THIS, c1=c1,
                per_core=per_core, shared=shared)


# --------------------------------------------------------------------------
# Device program
# --------------------------------------------------------------------------

def _build_program(T_LO, T_HI, c1, reps=1):
    TLOS, THIS = sum(T_LO), sum(T_HI)
    nc = bacc.Bacc("TRN2", target_bir_lowering=False, debug=False,
                   enable_asserts=False, num_devices=CORES)
    ap = {}

    def inp(name, shape, dt=DT.float32):
        ap[name] = nc.dram_tensor(name, list(shape), dt,
                                  kind="ExternalInput").ap()

    inp("x2", (N_NODES, F))
    inp("x1t", (DNN_IN, BSH))
    inp("idx", (128, TLOS), DT.int32)
    inp("nrm", (128, TLOS))
    inp("crl", (128, TLOS))
    inp("wslf", (128, NBLK))
    inp("w1", (F, F))
    inp("b1b", (128, F))
    inp("dnnw", (DNN_IN, F))
    inp("gma", (F, 1))
    inp("bta", (F, 1))
    inp("vc", (F, 1))
    inp("zc", (F, 1))
    inp("iota", (128, 128))
    out_ap = nc.dram_tensor("out", [1, BSH], DT.float32,
                            kind="ExternalOutput").ap()

    # chunk layout: groups of CHUNK_BLK dest blocks share one gather pair
    lo_off = np.concatenate([[0], np.cumsum(T_LO)]).astype(int)
    hi_off = np.concatenate([[0], np.cumsum(T_HI)]).astype(int)
    chunks = []
    for c0 in range(0, NBLK, CHUNK_BLK):
        c1b = min(c0 + CHUNK_BLK, NBLK)
        chunks.append(dict(
            blocks=list(range(c0, c1b)),
            lo_t0=int(lo_off[c0]), lo_t1=int(lo_off[c1b]),
            hi_t0=int(hi_off[c0]), hi_t1=int(hi_off[c1b]),
        ))
    max_lo = max(ch["lo_t1"] - ch["lo_t0"] for ch in chunks)
    max_hi = max(ch["hi_t1"] - ch["hi_t0"] for ch in chunks)

    with tile.TileContext(nc) as tc:
        with tc.tile_pool(name="const", bufs=1) as cp, \
             tc.tile_pool(name="dram", bufs=1, space="DRAM") as dp:
            # ---- load constants / per-core packed inputs once ----
            def load(name, shape, dt=DT.float32, src=None):
                t = cp.tile(list(shape), dt, tag=name)
                nc.sync.dma_start(out=t[:], in_=src if src is not None
                                  else ap[name][:])
                return t

            w1_sb = load("w1", (F, F))
            b1b_sb = load("b1b", (128, F))
            gma_sb = load("gma", (F, 1))
            bta_sb = load("bta", (F, 1))
            vc_sb = load("vc", (F, 1))
            zc_sb = load("zc", (F, 1))
            iota_sb = load("iota", (128, 128))
            wslf_sb = load("wslf", (128, NBLK))
            dnnw0 = load("dnnw0", (128, F), src=ap["dnnw"][0:128, :])
            dnnw1 = load("dnnw1", (128, F), src=ap["dnnw"][128:256, :])
            x1a = load("x1a", (128, BSH), src=ap["x1t"][0:128, :])
            x1b = load("x1b", (128, BSH), src=ap["x1t"][128:256, :])
            idx_sb = load("idx", (128, TLOS), DT.int32)
            nrm_sb = load("nrm", (128, TLOS))
            crl_sb = load("crl", (128, TLOS))

            ident = cp.tile([128, 128], DT.float32, tag="ident")
            make_identity(nc, ident[:])

            # persistent accumulators / phase outputs
            hT = cp.tile([F, BSH], DT.float32, tag="hT")
            sqtmp = cp.tile([F, BSH], DT.float32, tag="sqtmp")
            bn_sum = cp.tile([F, 1], DT.float32, tag="bn_sum")
            bn_sq = cp.tile([F, 1], DT.float32, tag="bn_sq")
            p_acc = cp.tile([F, 1], DT.float32, tag="p_acc")

            def phase_ab():
                if _PHASES["dnn"]:
                    _dnn_phase()
                else:
                    nc.vector.memset(hT[:], 0.0)
                    nc.vector.memset(bn_sum[:], 0.0)
                    nc.vector.memset(bn_sq[:], 1.0)
                    nc.vector.memset(sqtmp[:], 0.0)
                if _PHASES["gcn"]:
                    _gcn_phase()
                else:
                    nc.vector.memset(p_acc[:], 0.0)

            def _dnn_phase():
                with tc.tile_pool(name="pd", bufs=1, space="PSUM") as pd:
                    for c in range(BSH // 512):
                        ps = pd.tile([F, 512], DT.float32)
                        cs = slice(c * 512, (c + 1) * 512)
                        nc.tensor.matmul(out=ps[:], lhsT=dnnw0[:],
                                         rhs=x1a[:, cs], start=True, stop=False)
                        nc.tensor.matmul(out=ps[:], lhsT=dnnw1[:],
                                         rhs=x1b[:, cs], start=False, stop=True)
                        nc.vector.tensor_copy(out=hT[:, cs], in_=ps[:])
                nc.vector.reduce_sum(out=bn_sum[:], in_=hT[:],
                                     axis=mybir.AxisListType.X)
                nc.scalar.activation(out=sqtmp[:], in_=hT[:], func=ACTF.Square,
                                     accum_out=bn_sq[:])

            def _gcn_phase():
                # ---------------- GCN layer 1 ----------------
                nc.vector.memset(p_acc[:], 0.0)
                with tc.tile_pool(name="gp", bufs=2) as gp, \
                     tc.tile_pool(name="sp", bufs=4) as sp, \
                     tc.tile_pool(name="wp", bufs=3) as wp, \
                     tc.tile_pool(name="pa", bufs=2, space="PSUM") as pa, \
                     tc.tile_pool(name="pt", bufs=1, space="PSUM") as pt, \
                     tc.tile_pool(name="po", bufs=2, space="PSUM") as po, \
                     tc.tile_pool(name="pb", bufs=2, space="PSUM") as pb:
                    # gather groups: GMAX_T tiles (= one SWDGE ring's worth)
                    # per indirect call, one pool-buffered tile per group so
                    # in-flight gathers are capped at the pool's bufs
                    gtiles = {}

                    def get_group(g):
                        if g not in gtiles:
                            g0 = g * GMAX_T
                            nt = min(GMAX_T, TLOS - g0)
                            gt_ = gp.tile([128, GMAX_T * F], DT.float32,
                                          tag="glo")
                            if _PHASES["gather"]:
                                nc.gpsimd.indirect_dma_start(
                                    out=gt_[:, :nt * F],
                                    out_offset=None,
                                    in_=ap["x2"][:, :],
                                    in_offset=bass.IndirectOffsetOnAxis(
                                        ap=idx_sb[:, g0:g0 + nt], axis=0),
                                )
                            else:
                                nc.vector.memset(gt_[:], 0.0)
                            gtiles[g] = gt_
                        return gtiles[g]

                    for b in range(NBLK):
                        agg = pa.tile([128, F], DT.float32)
                        for ti in range(T_LO[b]):
                            gt = lo_off[b] + ti
                            gsrc = get_group(gt // GMAX_T)
                            lt = gt % GMAX_T
                            stile = sp.tile([128, 128], DT.float32,
                                            tag="stile")
                            nc.vector.tensor_scalar(
                                out=stile[:], in0=iota_sb[:],
                                scalar1=crl_sb[:, gt:gt + 1],
                                scalar2=nrm_sb[:, gt:gt + 1],
                                op0=ALU.is_equal, op1=ALU.mult)
                            nc.tensor.matmul(
                                out=agg[:], lhsT=stile[:],
                                rhs=gsrc[:, lt * F:(lt + 1) * F],
                                start=(ti == 0), stop=(ti == T_LO[b] - 1))
                        aggsb = wp.tile([128, F], DT.float32, tag="aggsb")
                        nc.vector.tensor_copy(out=aggsb[:], in_=agg[:])
                        pst = pt.tile([F, 128], DT.float32)
                        nc.tensor.transpose(out=pst[:], in_=aggsb[:],
                                            identity=ident[:])
                        aggT = wp.tile([F, 128], DT.float32, tag="aggT")
                        nc.vector.tensor_copy(out=aggT[:], in_=pst[:])
                        o1 = po.tile([128, F], DT.float32)
                        nc.tensor.matmul(out=o1[:], lhsT=aggT[:],
                                         rhs=w1_sb[:], start=True,
                                         stop=True)
                        g1 = wp.tile([128, F], DT.float32, tag="g1")
                        nc.vector.tensor_tensor(out=g1[:], in0=o1[:],
                                                in1=b1b_sb[:], op=ALU.add)
                        nc.scalar.activation(out=g1[:], in_=g1[:],
                                             func=ACTF.Relu)
                        pbt = pb.tile([F, 1], DT.float32)
                        nc.tensor.matmul(out=pbt[:], lhsT=g1[:],
                                         rhs=wslf_sb[:, b:b + 1],
                                         start=True, stop=True)
                        nc.vector.tensor_tensor(out=p_acc[:], in0=p_acc[:],
                                                in1=pbt[:], op=ALU.add)

            if reps == 1:
                phase_ab()
            else:
                with tc.For_i(0, reps, 1):
                    phase_ab()

            # ---------------- cross-core stats + head ----------------
            stats = cp.tile([F, 4], DT.float32, tag="stats")
            nc.vector.tensor_copy(out=stats[:, 0:1], in_=bn_sum[:])
            nc.vector.tensor_copy(out=stats[:, 1:2], in_=bn_sq[:])
            nc.vector.tensor_copy(out=stats[:, 2:3], in_=p_acc[:])
            cc_in = dp.tile([F, 3], DT.float32)
            cc_out = dp.tile([F, 3], DT.float32)
            nc.gpsimd.dma_start(out=cc_in[:], in_=stats[:, 0:3])
            nc.gpsimd.collective_compute(
                "AllReduce", ALU.add,
                replica_groups=[list(range(CORES))],
                ins=[cc_in.opt()], outs=[cc_out.opt()],
            )
            tot = cp.tile([F, 3], DT.float32, tag="tot")
            nc.sync.dma_start(out=tot[:], in_=cc_out[:])

            if not _PHASES["head"]:
                outsb0 = cp.tile([1, BSH], DT.float32, tag="outsb")
                nc.vector.memset(outsb0[:], 0.0)
                nc.sync.dma_start(out=out_ap[:], in_=outsb0[:])
            if _PHASES["head"]:
              with tc.tile_pool(name="pc", bufs=2, space="PSUM") as pc:
                sm = cp  # small [64,1] scratch tiles live in the const pool
                mu = sm.tile([F, 1], DT.float32, tag="mu")
                nc.vector.tensor_scalar(out=mu[:], in0=tot[:, 0:1],
                                        scalar1=1.0 / BATCH, scalar2=None,
                                        op0=ALU.mult)
                ex2 = sm.tile([F, 1], DT.float32, tag="ex2")
                nc.vector.tensor_scalar(out=ex2[:], in0=tot[:, 1:2],
                                        scalar1=1.0 / BATCH, scalar2=None,
                                        op0=ALU.mult)
                m2 = sm.tile([F, 1], DT.float32, tag="m2")
                nc.vector.tensor_tensor(out=m2[:], in0=mu[:], in1=mu[:],
                                        op=ALU.mult)
                var = sm.tile([F, 1], DT.float32, tag="var")
                nc.vector.tensor_tensor(out=var[:], in0=ex2[:], in1=m2[:],
                                        op=ALU.subtract)
                vp = sm.tile([F, 1], DT.float32, tag="vp")
                nc.vector.tensor_scalar(out=vp[:], in0=var[:],
                                        scalar1=BN_EPS, scalar2=None,
                                        op0=ALU.add)
                sd = sm.tile([F, 1], DT.float32, tag="sd")
                nc.scalar.activation(out=sd[:], in_=vp[:], func=ACTF.Sqrt)
                istd = sm.tile([F, 1], DT.float32, tag="istd")
                nc.vector.reciprocal(out=istd[:], in_=sd[:])
                scl = sm.tile([F, 1], DT.float32, tag="scl")
                nc.vector.tensor_tensor(out=scl[:], in0=istd[:], in1=gma_sb[:],
                                        op=ALU.mult)
                msc = sm.tile([F, 1], DT.float32, tag="msc")
                nc.vector.tensor_tensor(out=msc[:], in0=mu[:], in1=scl[:],
                                        op=ALU.mult)
                shf = sm.tile([F, 1], DT.float32, tag="shf")
                nc.vector.tensor_tensor(out=shf[:], in0=bta_sb[:], in1=msc[:],
                                        op=ALU.subtract)
                # dnn_embT = relu(hT * scale + shift), in place
                nc.scalar.activation(out=hT[:], in_=hT[:], func=ACTF.Relu,
                                     scale=scl[:, :], bias=shf[:, :])
                # s0 = (p . z)/N + c1
                s0p = pc.tile([1, 1], DT.float32, tag="s0p")
                nc.tensor.matmul(out=s0p[:], lhsT=zc_sb[:], rhs=tot[:, 2:3],
                                 start=True, stop=True)
                s0 = sm.tile([1, 1], DT.float32, tag="s0")
                nc.vector.tensor_scalar(out=s0[:], in0=s0p[:],
                                        scalar1=1.0 / N_NODES, scalar2=c1,
                                        op0=ALU.mult, op1=ALU.add)
                outsb = cp.tile([1, BSH], DT.float32, tag="outsb")
                for c in range(BSH // 512):
                    cs = slice(c * 512, (c + 1) * 512)
                    pov = pc.tile([1, 512], DT.float32, tag="pov")
                    nc.tensor.matmul(out=pov[:], lhsT=vc_sb[:], rhs=hT[:, cs],
                                     start=True, stop=True)
                    nc.vector.tensor_scalar(out=outsb[:, cs], in0=pov[:],
                                            scalar1=s0[:, :], scalar2=None,
                                            op0=ALU.add)
                nc.sync.dma_start(out=out_ap[:], in_=outsb[:])

    nc.compile()
    return nc


_CACHE = {}


def _get_program(T_LO, T_HI, c1, reps=1):
    key = (tuple(T_LO), tuple(T_HI), float(c1), reps)
    if key not in _CACHE:
        _CACHE[key] = _build_program(T_LO, T_HI, c1, reps)
    return _CACHE[key]


def _in_maps(st):
    maps = []
    for k in range(CORES):
        m = dict(st["shared"])
        m.update(st["per_core"][k])
        maps.append(m)
    return maps


def kernel(**inputs):
    st = _prep(inputs)
    nc = _get_program(st["T_LO"], st["T_HI"], st["c1"], reps=1)
    res = bass_utils.run_bass_kernel_spmd(
        nc, _in_maps(st), core_ids=list(range(CORES)))
    out = np.concatenate(
        [res.results[k]["out"].reshape(BSH, 1) for k in range(CORES)], axis=0)
    return out.astype(np.float32)


# revision 22
# speedup vs baseline: 22.5383x; 1.9633x over previous
"""Trainium2 Bass kernel: CombinedModel = DNN branch (Linear+BatchNorm+ReLU)
+ GCN branch (2x GCNConv -> mean pool) + linear head, on 8 NeuronCores.

Strategy (all FLOPs on float inputs run on-device):
- GCN layer 1: edges (incl. self-loops, sorted by destination) sharded by
  destination range across cores; per 128-edge tile, gather source x2 rows
  with indirect DMA (128 row-offsets per call, row-sorted within each dest
  block for HBM locality), build one-hot (dest) selection matrices on DVE,
  and scatter-accumulate on the TensorEngine into per-128-dest-block PSUM.
  aggX @ W1 (+b1, ReLU) per block.
- GCN layer 2 + mean pool collapse algebraically: only mean(out2) is needed,
  so sum_c out2[c] = (sum_n wslf[n]*g1[n]) @ W2 + N*b2, where wslf depends on
  the graph indices only (host-computed). Each core reduces its own dest
  shard; a single tiny AllReduce combines [bn_sum | bn_sumsq | p] stats.
- DNN branch: batch-sharded, computed transposed (hT = dnn_W^T @ x1^T) so
  BatchNorm scale/shift are per-partition; dnn_b cancels exactly in BN.
- Head: no nonlinearity between fc1/fc2 -> fold into out = dnn_emb @ v + s0
  with v = fc1_W[:64] @ fc2_W and s0 a scalar from the pooled GNN embedding.
"""

import math
import os
import sys

for _p in ("/opt/trn_rl_repo", "/root/.axon_site/_ro/trn_rl_repo"):
    if os.path.isdir(_p) and _p not in sys.path:
        sys.path.append(_p)

import numpy as np

import concourse.bacc as bacc
import concourse.bass as bass
import concourse.mybir as mybir
import concourse.tile as tile
from concourse import bass_utils
from concourse.masks import make_identity

DT = mybir.dt
ALU = mybir.AluOpType
ACTF = mybir.ActivationFunctionType

N_NODES = 50000
N_EDGES = 800000
BATCH = 16384
DNN_IN = 256
F = 64                       # feature width everywhere in the GNN
CORES = 8
NSH = N_NODES // CORES       # 6250 dest nodes per core
BSH = BATCH // CORES         # 2048 batch rows per core
NBLK = (NSH + 127) // 128    # 49 dest blocks per core
SPLIT = 32768                # int16 gather index limit
BN_EPS = 1e-5
CHUNK_BLK = 7                # dest blocks per gather chunk
GMAX_T = 1                   # tiles per indirect gather call: HW honors only
                             # one row-offset per partition per call

# debug switches: selectively disable program phases when bisecting
_PHASES = dict(dnn=True, gcn=True, head=True, gather=True, consts=True)


def _cdiv(a, b):
    return (a + b - 1) // b


# --------------------------------------------------------------------------
# Host-side preprocessing: graph indices -> per-core packed gather/one-hot
# metadata with a core-uniform tile structure (SPMD requires one program).
# --------------------------------------------------------------------------

def _prep(inputs):
    x1 = np.asarray(inputs["x1"], np.float32)
    x2 = np.ascontiguousarray(np.asarray(inputs["x2"], np.float32))
    ei = np.asarray(inputs["edge_index"])
    row = ei[0].astype(np.int64)
    col = ei[1].astype(np.int64)

    deg = (np.bincount(col, minlength=N_NODES) + 1.0).astype(np.float32)
    dis = (1.0 / np.sqrt(deg)).astype(np.float32)
    norm = dis[row] * dis[col]

    # layer-2 collapse weights: sum_c out2[c] = sum_n wslf[n] * h2[n] + N*b2
    w_r = np.bincount(row, weights=dis[col].astype(np.float64), minlength=N_NODES)
    wslf = (dis * w_r.astype(np.float32) + dis * dis).astype(np.float32)

    # self-loops as ordinary edges with norm = dis^2
    ar = np.arange(N_NODES, dtype=np.int64)
    row2 = np.concatenate([row, ar])
    col2 = np.concatenate([col, ar])
    nrm2 = np.concatenate([norm, dis * dis]).astype(np.float32)

    order = np.argsort(col2, kind="stable")
    srow = row2[order]
    scol = col2[order]
    snrm = nrm2[order]

    # per (core, block) segments; indirect DMA takes int32 row offsets, so
    # no index-range splitting is needed
    segs = [[None] * NBLK for _ in range(CORES)]
    n_e = np.zeros((CORES, NBLK), np.int64)
    for k in range(CORES):
        base = k * NSH
        s0 = np.searchsorted(scol, base)
        s1 = np.searchsorted(scol, base + NSH)
        krow = srow[s0:s1]
        knrm = snrm[s0:s1]
        rel = scol[s0:s1] - base
        bst = np.searchsorted(rel, np.arange(NBLK) * 128)
        ben = np.append(bst[1:], rel.size)
        for b in range(NBLK):
            sl = slice(bst[b], ben[b])
            r = krow[sl]
            n = knrm[sl]
            c = (rel[sl] - b * 128).astype(np.float32)
            # within a block the edge order is irrelevant to the sum; sort
            # by source row so the gather's HBM accesses are ascending
            o = np.argsort(r, kind="stable")
            segs[k][b] = (r[o], n[o], c[o])
            n_e[k, b] = ben[b] - bst[b]

    T_LO = [int(_cdiv(int(n_e[:, b].max()), 128)) for b in range(NBLK)]
    T_HI = [0] * NBLK
    for b in range(NBLK):
        assert T_LO[b] >= 1
    TLOS = sum(T_LO)
    THIS = 0

    def pack_stream(k):
        # concatenated per-block edge data padded to T[b]*128 entries
        tot = TLOS * 128
        idx = np.zeros(tot, np.int32)
        nrm = np.zeros(tot, np.float32)
        crl = np.zeros(tot, np.float32)
        off = 0
        for b in range(NBLK):
            r, n, c = segs[k][b]
            m = r.size
            idx[off:off + m] = r.astype(np.int32)
            nrm[off:off + m] = n
            crl[off:off + m] = c
            off += T_LO[b] * 128
        ntile = tot // 128
        # [128, T] layout: column t, partition p  <->  edge t*128 + p
        idx_t = np.ascontiguousarray(idx.reshape(ntile, 128).T)
        nrm_t = np.ascontiguousarray(nrm.reshape(ntile, 128).T)
        crl_t = np.ascontiguousarray(crl.reshape(ntile, 128).T)
        return idx_t, nrm_t, crl_t

    per_core = []
    x1t_full = np.ascontiguousarray(x1.T)
    for k in range(CORES):
        ilo, nlo, clo = pack_stream(k)
        wk = np.zeros(NBLK * 128, np.float32)
        wk[:NSH] = wslf[k * NSH:(k + 1) * NSH]
        per_core.append(dict(
            idx=ilo, nrm=nlo, crl=clo,
            wslf=np.ascontiguousarray(wk.reshape(NBLK, 128).T),
            x1t=np.ascontiguousarray(x1t_full[:, k * BSH:(k + 1) * BSH]),
        ))

    # host-folded head weights (no nonlinearity between fc1 and fc2)
    fc1 = np.asarray(inputs["fc1_W"], np.float32)
    fc2 = np.asarray(inputs["fc2_W"], np.float32)
    u = fc1[F:, :] @ fc2                                    # [64, 1]
    v = np.ascontiguousarray(fc1[:F, :] @ fc2)              # [64, 1]
    z = np.ascontiguousarray(np.asarray(inputs["gcn2_W"], np.float32) @ u)
    c1 = float(np.asarray(inputs["fc1_b"], np.float32) @ fc2[:, 0]
               + np.asarray(inputs["fc2_b"], np.float32)[0]
               + np.asarray(inputs["gcn2_b"], np.float32) @ u[:, 0])

    shared = dict(
        x2=x2,
        w1=np.ascontiguousarray(np.asarray(inputs["gcn1_W"], np.float32)),
        b1b=np.ascontiguousarray(
            np.tile(np.asarray(inputs["gcn1_b"], np.float32), (128, 1))),
        dnnw=np.ascontiguousarray(np.asarray(inputs["dnn_W"], np.float32)),
        gma=np.ascontiguousarray(
            np.asarray(inputs["bn_gamma"], np.float32).reshape(F, 1)),
        bta=np.ascontiguousarray(
            np.asarray(inputs["bn_beta"], np.float32).reshape(F, 1)),
        vc=v, zc=z,
        iota=np.ascontiguousarray(
            np.broadcast_to(np.arange(128, dtype=np.float32), (128, 128))),
    )
    return dict(T_LO=T_LO, T_HI=T_HI, TLOS=TLOS, THIS=THIS, c1=c1,
                per_core=per_core, shared=shared)


# --------------------------------------------------------------------------
# Device program
# --------------------------------------------------------------------------

def _build_program(T_LO, T_HI, c1, reps=1):
    TLOS, THIS = sum(T_LO), sum(T_HI)
    nc = bacc.Bacc("TRN2", target_bir_lowering=False, debug=False,
                   enable_asserts=False, num_devices=CORES)
    ap = {}

    def inp(name, shape, dt=DT.float32):
        ap[name] = nc.dram_tensor(name, list(shape), dt,
                                  kind="ExternalInput").ap()

    inp("x2", (N_NODES, F))
    inp("x1t", (DNN_IN, BSH))
    inp("idx", (128, TLOS), DT.int32)
    inp("nrm", (128, TLOS))
    inp("crl", (128, TLOS))
    inp("wslf", (128, NBLK))
    inp("w1", (F, F))
    inp("b1b", (128, F))
    inp("dnnw", (DNN_IN, F))
    inp("gma", (F, 1))
    inp("bta", (F, 1))
    inp("vc", (F, 1))
    inp("zc", (F, 1))
    inp("iota", (128, 128))
    out_ap = nc.dram_tensor("out", [1, BSH], DT.float32,
                            kind="ExternalOutput").ap()

    # chunk layout: groups of CHUNK_BLK dest blocks share one gather pair
    lo_off = np.concatenate([[0], np.cumsum(T_LO)]).astype(int)
    hi_off = np.concatenate([[0], np.cumsum(T_HI)]).astype(int)
    chunks = []
    for c0 in range(0, NBLK, CHUNK_BLK):
        c1b = min(c0 + CHUNK_BLK, NBLK)
        chunks.append(dict(
            blocks=list(range(c0, c1b)),
            lo_t0=int(lo_off[c0]), lo_t1=int(lo_off[c1b]),
            hi_t0=int(hi_off[c0]), hi_t1=int(hi_off[c1b]),
        ))
    max_lo = max(ch["lo_t1"] - ch["lo_t0"] for ch in chunks)
    max_hi = max(ch["hi_t1"] - ch["hi_t0"] for ch in chunks)

    with tile.TileContext(nc) as tc:
        with tc.tile_pool(name="const", bufs=1) as cp, \
             tc.tile_pool(name="dram", bufs=1, space="DRAM") as dp:
            # ---- load constants / per-core packed inputs once ----
            def load(name, shape, dt=DT.float32, src=None):
                t = cp.tile(list(shape), dt, tag=name)
                nc.sync.dma_start(out=t[:], in_=src if src is not None
                                  else ap[name][:])
                return t

            w1_sb = load("w1", (F, F))
            b1b_sb = load("b1b", (128, F))
            gma_sb = load("gma", (F, 1))
            bta_sb = load("bta", (F, 1))
            vc_sb = load("vc", (F, 1))
            zc_sb = load("zc", (F, 1))
            iota_sb = load("iota", (128, 128))
            wslf_sb = load("wslf", (128, NBLK))
            dnnw0 = load("dnnw0", (128, F), src=ap["dnnw"][0:128, :])
            dnnw1 = load("dnnw1", (128, F), src=ap["dnnw"][128:256, :])
            x1a = load("x1a", (128, BSH), src=ap["x1t"][0:128, :])
            x1b = load("x1b", (128, BSH), src=ap["x1t"][128:256, :])
            idx_sb = load("idx", (128, TLOS), DT.int32)
            nrm_sb = load("nrm", (128, TLOS))
            crl_sb = load("crl", (128, TLOS))

            ident = cp.tile([128, 128], DT.float32, tag="ident")
            make_identity(nc, ident[:])

            # persistent accumulators / phase outputs
            hT = cp.tile([F, BSH], DT.float32, tag="hT")
            sqtmp = cp.tile([F, BSH], DT.float32, tag="sqtmp")
            bn_sum = cp.tile([F, 1], DT.float32, tag="bn_sum")
            bn_sq = cp.tile([F, 1], DT.float32, tag="bn_sq")
            p_acc = cp.tile([F, 1], DT.float32, tag="p_acc")

            def phase_ab():
                if _PHASES["dnn"]:
                    _dnn_phase()
                else:
                    nc.vector.memset(hT[:], 0.0)
                    nc.vector.memset(bn_sum[:], 0.0)
                    nc.vector.memset(bn_sq[:], 1.0)
                    nc.vector.memset(sqtmp[:], 0.0)
                if _PHASES["gcn"]:
                    _gcn_phase()
                else:
                    nc.vector.memset(p_acc[:], 0.0)

            def _dnn_phase():
                with tc.tile_pool(name="pd", bufs=1, space="PSUM") as pd:
                    for c in range(BSH // 512):
                        ps = pd.tile([F, 512], DT.float32)
                        cs = slice(c * 512, (c + 1) * 512)
                        nc.tensor.matmul(out=ps[:], lhsT=dnnw0[:],
                                         rhs=x1a[:, cs], start=True, stop=False)
                        nc.tensor.matmul(out=ps[:], lhsT=dnnw1[:],
                                         rhs=x1b[:, cs], start=False, stop=True)
                        nc.vector.tensor_copy(out=hT[:, cs], in_=ps[:])
                nc.vector.reduce_sum(out=bn_sum[:], in_=hT[:],
                                     axis=mybir.AxisListType.X)
                nc.scalar.activation(out=sqtmp[:], in_=hT[:], func=ACTF.Square,
                                     accum_out=bn_sq[:])

            def _gcn_phase():
                # ---------------- GCN layer 1 ----------------
                nc.vector.memset(p_acc[:], 0.0)
                with tc.tile_pool(name="gp", bufs=2) as gp, \
                     tc.tile_pool(name="sp", bufs=4) as sp, \
                     tc.tile_pool(name="wp", bufs=3) as wp, \
                     tc.tile_pool(name="pa", bufs=2, space="PSUM") as pa, \
                     tc.tile_pool(name="pt", bufs=1, space="PSUM") as pt, \
                     tc.tile_pool(name="po", bufs=2, space="PSUM") as po, \
                     tc.tile_pool(name="pb", bufs=2, space="PSUM") as pb:
                    # gather groups: GMAX_T tiles (= one SWDGE ring's worth)
                    # per indirect call, one pool-buffered tile per group so
                    # in-flight gathers are capped at the pool's bufs
                    gtiles = {}

                    def get_group(g):
                        if g not in gtiles:
                            g0 = g * GMAX_T
                            nt = min(GMAX_T, TLOS - g0)
                            gt_ = gp.tile([128, GMAX_T * F], DT.float32,
                                          tag="glo")
                            if _PHASES["gather"]:
                                nc.gpsimd.indirect_dma_start(
                                    out=gt_[:, :nt * F],
                                    out_offset=None,
                                    in_=ap["x2"][:, :],
                                    in_offset=bass.IndirectOffsetOnAxis(
                                        ap=idx_sb[:, g0:g0 + nt], axis=0),
                                )
                            else:
                                nc.vector.memset(gt_[:], 0.0)
                            gtiles[g] = gt_
                        return gtiles[g]

                    for b in range(NBLK):
                        agg = pa.tile([128, F], DT.float32)
                        for ti in range(T_LO[b]):
                            gt = lo_off[b] + ti
                            gsrc = get_group(gt // GMAX_T)
                            lt = gt % GMAX_T
                            stile = sp.tile([128, 128], DT.float32,
                                            tag="stile")
                            nc.vector.tensor_scalar(
                                out=stile[:], in0=iota_sb[:],
                                scalar1=crl_sb[:, gt:gt + 1],
                                scalar2=nrm_sb[:, gt:gt + 1],
                                op0=ALU.is_equal, op1=ALU.mult)
                            nc.tensor.matmul(
                                out=agg[:], lhsT=stile[:],
                                rhs=gsrc[:, lt * F:(lt + 1) * F],
                                start=(ti == 0), stop=(ti == T_LO[b] - 1))
                        aggsb = wp.tile([128, F], DT.float32, tag="aggsb")
                        nc.vector.tensor_copy(out=aggsb[:], in_=agg[:])
                        pst = pt.tile([F, 128], DT.float32)
                        nc.tensor.transpose(out=pst[:], in_=aggsb[:],
                                            identity=ident[:])
                        aggT = wp.tile([F, 128], DT.float32, tag="aggT")
                        nc.vector.tensor_copy(out=aggT[:], in_=pst[:])
                        o1 = po.tile([128, F], DT.float32)
                        nc.tensor.matmul(out=o1[:], lhsT=aggT[:],
                                         rhs=w1_sb[:], start=True,
                                         stop=True)
                        g1 = wp.tile([128, F], DT.float32, tag="g1")
                        nc.vector.tensor_tensor(out=g1[:], in0=o1[:],
                                                in1=b1b_sb[:], op=ALU.add)
                        nc.scalar.activation(out=g1[:], in_=g1[:],
                                             func=ACTF.Relu)
                        pbt = pb.tile([F, 1], DT.float32)
                        nc.tensor.matmul(out=pbt[:], lhsT=g1[:],
                                         rhs=wslf_sb[:, b:b + 1],
                                         start=True, stop=True)
                        nc.vector.tensor_tensor(out=p_acc[:], in0=p_acc[:],
                                                in1=pbt[:], op=ALU.add)

            if reps == 1:
                phase_ab()
            else:
                with tc.For_i(0, reps, 1):
                    phase_ab()

            # ---------------- cross-core stats + head ----------------
            stats = cp.tile([F, 4], DT.float32, tag="stats")
            nc.vector.tensor_copy(out=stats[:, 0:1], in_=bn_sum[:])
            nc.vector.tensor_copy(out=stats[:, 1:2], in_=bn_sq[:])
            nc.vector.tensor_copy(out=stats[:, 2:3], in_=p_acc[:])
            cc_in = dp.tile([F, 3], DT.float32)
            cc_out = dp.tile([F, 3], DT.float32)
            nc.gpsimd.dma_start(out=cc_in[:], in_=stats[:, 0:3])
            nc.gpsimd.collective_compute(
                "AllReduce", ALU.add,
                replica_groups=[list(range(CORES))],
                ins=[cc_in.opt()], outs=[cc_out.opt()],
            )
            tot = cp.tile([F, 3], DT.float32, tag="tot")
            nc.sync.dma_start(out=tot[:], in_=cc_out[:])

            if not _PHASES["head"]:
                outsb0 = cp.tile([1, BSH], DT.float32, tag="outsb")
                nc.vector.memset(outsb0[:], 0.0)
                nc.sync.dma_start(out=out_ap[:], in_=outsb0[:])
            if _PHASES["head"]:
              with tc.tile_pool(name="pc", bufs=2, space="PSUM") as pc:
                sm = cp  # small [64,1] scratch tiles live in the const pool
                mu = sm.tile([F, 1], DT.float32, tag="mu")
                nc.vector.tensor_scalar(out=mu[:], in0=tot[:, 0:1],
                                        scalar1=1.0 / BATCH, scalar2=None,
                                        op0=ALU.mult)
                ex2 = sm.tile([F, 1], DT.float32, tag="ex2")
                nc.vector.tensor_scalar(out=ex2[:], in0=tot[:, 1:2],
                                        scalar1=1.0 / BATCH, scalar2=None,
                                        op0=ALU.mult)
                m2 = sm.tile([F, 1], DT.float32, tag="m2")
                nc.vector.tensor_tensor(out=m2[:], in0=mu[:], in1=mu[:],
                                        op=ALU.mult)
                var = sm.tile([F, 1], DT.float32, tag="var")
                nc.vector.tensor_tensor(out=var[:], in0=ex2[:], in1=m2[:],
                                        op=ALU.subtract)
                vp = sm.tile([F, 1], DT.float32, tag="vp")
                nc.vector.tensor_scalar(out=vp[:], in0=var[:],
                                        scalar1=BN_EPS, scalar2=None,
                                        op0=ALU.add)
                sd = sm.tile([F, 1], DT.float32, tag="sd")
                nc.scalar.activation(out=sd[:], in_=vp[:], func=ACTF.Sqrt)
                istd = sm.tile([F, 1], DT.float32, tag="istd")
                nc.vector.reciprocal(out=istd[:], in_=sd[:])
                scl = sm.tile([F, 1], DT.float32, tag="scl")
                nc.vector.tensor_tensor(out=scl[:], in0=istd[:], in1=gma_sb[:],
                                        op=ALU.mult)
                msc = sm.tile([F, 1], DT.float32, tag="msc")
                nc.vector.tensor_tensor(out=msc[:], in0=mu[:], in1=scl[:],
                                        op=ALU.mult)
                shf = sm.tile([F, 1], DT.float32, tag="shf")
                nc.vector.tensor_tensor(out=shf[:], in0=bta_sb[:], in1=msc[:],
                                        op=ALU.subtract)
                # dnn_embT = relu(hT * scale + shift), in place
                nc.scalar.activation(out=hT[:], in_=hT[:], func=ACTF.Relu,
                                     scale=scl[:, :], bias=shf[:, :])
                # s0 = (p . z)/N + c1
                s0p = pc.tile([1, 1], DT.float32, tag="s0p")
                nc.tensor.matmul(out=s0p[:], lhsT=zc_sb[:], rhs=tot[:, 2:3],
                                 start=True, stop=True)
                s0 = sm.tile([1, 1], DT.float32, tag="s0")
                nc.vector.tensor_scalar(out=s0[:], in0=s0p[:],
                                        scalar1=1.0 / N_NODES, scalar2=c1,
                                        op0=ALU.mult, op1=ALU.add)
                outsb = cp.tile([1, BSH], DT.float32, tag="outsb")
                for c in range(BSH // 512):
                    cs = slice(c * 512, (c + 1) * 512)
                    pov = pc.tile([1, 512], DT.float32, tag="pov")
                    nc.tensor.matmul(out=pov[:], lhsT=vc_sb[:], rhs=hT[:, cs],
                                     start=True, stop=True)
                    nc.vector.tensor_scalar(out=outsb[:, cs], in0=pov[:],
                                            scalar1=s0[:, :], scalar2=None,
                                            op0=ALU.add)
                nc.sync.dma_start(out=out_ap[:], in_=outsb[:])

    nc.compile()
    return nc


_CACHE = {}


def _get_program(T_LO, T_HI, c1, reps=1):
    key = (tuple(T_LO), tuple(T_HI), float(c1), reps)
    if key not in _CACHE:
        _CACHE[key] = _build_program(T_LO, T_HI, c1, reps)
    return _CACHE[key]


def _in_maps(st):
    maps = []
    for k in range(CORES):
        m = dict(st["shared"])
        m.update(st["per_core"][k])
        maps.append(m)
    return maps


def kernel(**inputs):
    st = _prep(inputs)
    nc = _get_program(st["T_LO"], st["T_HI"], st["c1"], reps=1)
    res = bass_utils.run_bass_kernel_spmd(
        nc, _in_maps(st), core_ids=list(range(CORES)))
    out = np.concatenate(
        [res.results[k]["out"].reshape(BSH, 1) for k in range(CORES)], axis=0)
    return out.astype(np.float32)
